# revision 5
# baseline (speedup 1.0000x reference)
"""GQA attention (B=2, T=2048, D=2048, H=16, HK=4, HD=128) on 8 TRN2 NeuronCores.

Sharding: core = (b, g) for b in {0,1}, g in {0..3}: each core handles one batch
element and one kv head with its group of 4 q heads. Each core computes its
partial output contribution x_b @ Wq_g ... @ Wo_g -> [T, D]; host sums the 4
partials per batch element.

Device dataflow (per core), all big matmuls in bf16 with fp32 PSUM accumulation:
  qT_h [d=128, T] = Wq_h.T @ x.T      (4 heads)     } RoPE applied in fp32 via a
  kT   [d=128, T] = Wk.T @ x.T                      } pair-swap matmul (PE) + DVE
  v    [T, 128]   = x @ Wv            (natural layout, + ones column -> v_aug)
  scoresT [s,q]   = kT_chunk.T-as-lhsT ... = k @ qT  (contraction over d)
  probsT  [s,q]   = exp(scoresT)  (no max subtraction -- scores are O(5))
  out_aug [q,129] = probsT.T @ v_aug   (col 128 = softmax denominator)
  out_n   [q,128] = out_aug[:, :128] * recip(out_aug[:, 128])
  oT      [d, q]  = PE-transpose(out_n)
  partial [T, D]  = oT.T @ Wo_g        (accumulate 4 head chunks)
"""

import sys

if "/opt/trn_rl_repo" not in sys.path:
    sys.path.insert(0, "/opt/trn_rl_repo")

from contextlib import ExitStack

import ml_dtypes
import numpy as np

import concourse.bacc as bacc
import concourse.tile as tile
from concourse import mybir
from concourse.bass_utils import run_bass_kernel_spmd

BF = ml_dtypes.bfloat16

B, T, D = 2, 2048, 2048
H, HK, HD = 16, 4, 128
REP = H // HK  # q heads per kv head (= heads per core)
P = 128
KC = D // P    # contraction chunks for the projections
NT = T // P    # 128-row tiles of T
NQB = T // 512 # 512-wide q blocks

_CACHE = {}


def _build(causal: bool):
    bf = mybir.dt.bfloat16
    f32 = mybir.dt.float32
    nc = bacc.Bacc("TRN2", target_bir_lowering=False, debug=False,
                   enable_asserts=False)

    xT = nc.dram_tensor("xT", [D, T], bf, kind="ExternalInput").ap()
    wq = nc.dram_tensor("wq", [D, REP * HD], bf, kind="ExternalInput").ap()
    wk = nc.dram_tensor("wk", [D, HD], bf, kind="ExternalInput").ap()
    wv = nc.dram_tensor("wv", [D, HD], bf, kind="ExternalInput").ap()
    wo = nc.dram_tensor("wo", [REP * HD, D], bf, kind="ExternalInput").ap()
    cos = nc.dram_tensor("cose", [P, T], f32, kind="ExternalInput").ap()
    sin = nc.dram_tensor("sine", [P, T], f32, kind="ExternalInput").ap()
    mt = nc.dram_tensor("mt", [P, P], f32, kind="ExternalInput").ap()
    idn = nc.dram_tensor("idn", [P, P], bf, kind="ExternalInput").ap()
    if causal:
        masks = nc.dram_tensor("masks", [P, 4 * 512], bf,
                               kind="ExternalInput").ap()
    else:
        maskT = nc.dram_tensor("maskT", [T, T], bf, kind="ExternalInput").ap()
    out = nc.dram_tensor("out", [T, D], f32, kind="ExternalOutput").ap()

    EXP = mybir.ActivationFunctionType.Exp

    with tile.TileContext(nc) as tc, ExitStack() as ctx:
        singles = ctx.enter_context(tc.tile_pool(name="singles", bufs=1))
        ps = ctx.enter_context(tc.tile_pool(name="ps", bufs=8, space="PSUM"))
        sb_raw = ctx.enter_context(tc.tile_pool(name="raw", bufs=3))
        sb_tmp = ctx.enter_context(tc.tile_pool(name="tmp", bufs=4))
        sb_probs = ctx.enter_context(tc.tile_pool(name="probs", bufs=6))
        sb_small = ctx.enter_context(tc.tile_pool(name="small", bufs=4))
        sb_out = ctx.enter_context(tc.tile_pool(name="outst", bufs=3))
        if not causal:
            sb_mask = ctx.enter_context(tc.tile_pool(name="mask", bufs=18))

        # ---- resident inputs (chunked so compute can start per-chunk) ----
        xT_c, wq_c, wk_c, wv_c = [], [], [], []
        for c in range(KC):
            t_ = singles.tile([P, T], bf, tag=f"xT{c}")
            nc.sync.dma_start(out=t_, in_=xT[c * P:(c + 1) * P, :])
            xT_c.append(t_)
            t_ = singles.tile([P, REP * HD], bf, tag=f"wq{c}")
            nc.sync.dma_start(out=t_, in_=wq[c * P:(c + 1) * P, :])
            wq_c.append(t_)
            t_ = singles.tile([P, HD], bf, tag=f"wk{c}")
            nc.sync.dma_start(out=t_, in_=wk[c * P:(c + 1) * P, :])
            wk_c.append(t_)
            t_ = singles.tile([P, HD], bf, tag=f"wv{c}")
            nc.sync.dma_start(out=t_, in_=wv[c * P:(c + 1) * P, :])
            wv_c.append(t_)
        wo_h = []
        for h in range(REP):
            t_ = singles.tile([P, D], bf, tag=f"wo{h}")
            nc.sync.dma_start(out=t_, in_=wo[h * P:(h + 1) * P, :])
            wo_h.append(t_)
        cos_sb = singles.tile([P, T], f32, tag="cos")
        nc.sync.dma_start(out=cos_sb, in_=cos)
        sin_sb = singles.tile([P, T], f32, tag="sin")
        nc.sync.dma_start(out=sin_sb, in_=sin)
        mt_sb = singles.tile([P, P], f32, tag="mt")
        nc.sync.dma_start(out=mt_sb, in_=mt)
        id_sb = singles.tile([P, P], bf, tag="idn")
        nc.sync.dma_start(out=id_sb, in_=idn)
        if causal:
            masks_sb = singles.tile([P, 4, 512], bf, tag="masks")
            nc.sync.dma_start(out=masks_sb, in_=masks.rearrange(
                "p (r n) -> p r n", r=4))

        qT = singles.tile([P, REP, T], bf, tag="qT")
        kT = singles.tile([P, T], bf, tag="kT")
        vax = singles.tile([P, NT, HD + 1], bf, tag="vax")
        oT = singles.tile([P, REP, T], bf, tag="oT")

        # ---- projections with RoPE (qT, kT), v natural + ones column ----
        def proj_rope(dst, lhsT_of):
            # dst: bf16 [P, T] slice target; lhsT_of(c) -> [P(Dchunk), 128]
            for nb in range(T // 512):
                sl = slice(nb * 512, (nb + 1) * 512)
                pt = ps.tile([P, 512], f32, tag="ps")
                for c in range(KC):
                    nc.tensor.matmul(pt, lhsT=lhsT_of(c),
                                     rhs=xT_c[c][:, sl],
                                     start=(c == 0), stop=(c == KC - 1))
                raw = sb_raw.tile([P, 512], f32, tag="raw")
                nc.vector.tensor_copy(raw, pt)
                sh = ps.tile([P, 512], f32, tag="ps")
                nc.tensor.matmul(sh, lhsT=mt_sb, rhs=raw, start=True,
                                 stop=True)
                ta = sb_tmp.tile([P, 512], f32, tag="tmp")
                nc.vector.tensor_mul(ta, raw, cos_sb[:, sl])
                tb = sb_tmp.tile([P, 512], f32, tag="tmp")
                nc.vector.tensor_mul(tb, sh, sin_sb[:, sl])
                nc.vector.tensor_add(dst[:, sl], ta, tb)

        for h in range(REP):
            proj_rope(qT[:, h, :],
                      lambda c, h=h: wq_c[c][:, h * HD:(h + 1) * HD])
        proj_rope(kT, lambda c: wk_c[c])

        for m in range(NT):
            pv = ps.tile([P, 512], f32, tag="ps")
            for c in range(KC):
                nc.tensor.matmul(pv[:, :HD],
                                 lhsT=xT_c[c][:, m * P:(m + 1) * P],
                                 rhs=wv_c[c],
                                 start=(c == 0), stop=(c == KC - 1))
            nc.vector.tensor_copy(vax[:, m, :HD], pv[:, :HD])
        nc.vector.memset(vax[:, :, HD], 1.0)

        # ---- attention ----
        for qb in range(NQB):
            qsl = slice(qb * 512, (qb + 1) * 512)
            nj = 4 * qb + 4 if causal else NT
            if not causal:
                mts = []
                for j in range(nj):
                    t_ = sb_mask.tile([P, 512], bf, tag="maskt")
                    nc.sync.dma_start(
                        out=t_, in_=maskT[j * P:(j + 1) * P, qsl])
                    mts.append(t_)
            for h in range(REP):
                oaug = [ps.tile([P, HD + 1], f32, tag="ps",
                                name=f"oaug{qb}_{h}_{k}")
                        for k in range(4)]
                for j in range(nj):
                    sc = ps.tile([P, 512], f32, tag="ps")
                    nc.tensor.matmul(sc, lhsT=kT[:, j * P:(j + 1) * P],
                                     rhs=qT[:, h, qsl],
                                     start=True, stop=True)
                    if not causal:
                        nc.vector.tensor_add(sc, sc, mts[j])
                    pr = sb_probs.tile([P, 512], bf, tag="probs")
                    nc.scalar.activation(pr, sc, EXP)
                    r = j - 4 * qb
                    if causal and r >= 0:
                        nc.vector.tensor_mul(pr, pr, masks_sb[:, r, :])
                    for mi in range(4):
                        m = qb * 4 + mi
                        if causal and j > m:
                            continue
                        last = (j == m) if causal else (j == nj - 1)
                        nc.tensor.matmul(oaug[mi],
                                         lhsT=pr[:, mi * P:(mi + 1) * P],
                                         rhs=vax[:, j, :],
                                         start=(j == 0), stop=last)
                for mi in range(4):
                    m = qb * 4 + mi
                    rec = sb_small.tile([P, 1], f32, tag="rec")
                    nc.vector.reciprocal(rec, oaug[mi][:, HD:HD + 1])
                    on = sb_small.tile([P, HD], bf, tag="onrm")
                    nc.vector.tensor_scalar_mul(on, oaug[mi][:, :HD], rec)
                    tp = ps.tile([P, P], bf, tag="ps")
                    nc.tensor.transpose(tp, on, id_sb)
                    nc.vector.tensor_copy(oT[:, h, m * P:(m + 1) * P], tp)

        # ---- output projection ----
        for m in range(NT):
            for n in range(D // 512):
                wops = ps.tile([P, 512], f32, tag="ps")
                for h in range(REP):
                    nc.tensor.matmul(wops,
                                     lhsT=oT[:, h, m * P:(m + 1) * P],
                                     rhs=wo_h[h][:, n * 512:(n + 1) * 512],
                                     start=(h == 0), stop=(h == REP - 1))
                ost = sb_out.tile([P, 512], f32, tag="outst")
                nc.scalar.copy(ost, wops)
                nc.sync.dma_start(
                    out=out[m * P:(m + 1) * P, n * 512:(n + 1) * 512],
                    in_=ost)

    nc.compile()
    return nc


def _get(causal: bool):
    if causal not in _CACHE:
        _CACHE[causal] = _build(causal)
    return _CACHE[causal]


def _is_causal(mask: np.ndarray) -> bool:
    if mask.shape != (T, T):
        return False
    tril = np.tril(np.ones((T, T), dtype=bool))
    if not np.all(mask[tril] == 0.0):
        return False
    return bool(np.all(np.isneginf(mask[~tril])))


def kernel(x, freqs_cos, freqs_sin, mask, wq, wk, wv, wo):
    causal = _is_causal(np.asarray(mask))
    nc = _get(causal)

    scale = np.float32(1.0 / np.sqrt(HD))
    cos_e = np.repeat(np.ascontiguousarray(freqs_cos.T), 2, axis=0).astype(
        np.float32)
    sin_e = np.repeat(np.ascontiguousarray(freqs_sin.T), 2, axis=0).astype(
        np.float32)
    mt = np.zeros((P, P), np.float32)
    for i in range(P // 2):
        mt[2 * i + 1, 2 * i] = -1.0  # shuf[2i]   = -q[2i+1]
        mt[2 * i, 2 * i + 1] = 1.0   # shuf[2i+1] = +q[2i]
    idn = np.eye(P, dtype=BF)
    if causal:
        s_i = np.arange(P)[:, None]
        q_i = np.arange(512)[None, :]
        m_r = np.stack(
            [(r * P + s_i <= q_i) for r in range(4)], axis=1).astype(BF)
        masks_h = np.ascontiguousarray(m_r.reshape(P, 4 * 512))
    else:
        maskT_h = np.ascontiguousarray(mask.T).astype(BF)

    xT_b = [np.ascontiguousarray(x[b].T).astype(BF) for b in range(B)]
    in_maps = []
    for b in range(B):
        for g in range(HK):
            m = {
                "xT": xT_b[b],
                "wq": (wq[:, g * REP * HD:(g + 1) * REP * HD]
                       * scale).astype(BF),
                "wk": wk[:, g * HD:(g + 1) * HD].astype(BF),
                "wv": wv[:, g * HD:(g + 1) * HD].astype(BF),
                "wo": wo[g * REP * HD:(g + 1) * REP * HD, :].astype(BF),
                "cose": cos_e, "sine": sin_e, "mt": mt, "idn": idn,
            }
            if causal:
                m["masks"] = masks_h
            else:
                m["maskT"] = maskT_h
            in_maps.append(m)

    res = run_bass_kernel_spmd(nc, in_maps, core_ids=list(range(B * HK)))
    full = np.zeros((B, T, D), np.float32)
    for b in range(B):
        for g in range(HK):
            full[b] += res.results[b * HK + g]["out"]
    return full


# revision 11
# speedup vs baseline: 2.2223x; 2.2223x over previous
"""GQA attention (B=2, T=2048, D=2048, H=16, HK=4, HD=128) on 8 TRN2 NeuronCores.

Sharding: core = (b, g) for b in {0,1}, g in {0..3}: each core handles one batch
element and one kv head with its group of 4 q heads. Each core computes its
partial output contribution x_b @ Wq_g ... @ Wo_g -> [T, D]; host sums the 4
partials per batch element.

Device dataflow (per core), all big matmuls in bf16 with fp32 PSUM accumulation:
  qT_h [d=128, T] = Wq_h.T @ x.T      (4 heads)     } RoPE applied in fp32 via a
  kT   [d=128, T] = Wk.T @ x.T                      } pair-swap matmul (PE) + DVE
  v    [T, 128]   = x @ Wv            (natural layout, + ones column -> v_aug)
  scoresT [s,q]   = k @ qT            (contraction over d; s on partitions)
  probsT  [s,q]   = exp(scoresT)      (no max subtraction -- scores are O(5))
  out_aug [q,129] = probsT.T @ v_aug  (col 128 = softmax denominator)
  out_n   [q,128] = out_aug[:, :128] * recip(out_aug[:, 128])
  oT      [d, q]  = PE-transpose(out_n)
  partial [T, D]  = oT.T @ Wo_g       (accumulate 4 head chunks)

The kernel is one fused pipeline over 512-wide query blocks (qb): each qb
iteration projects its slice of q/k/v, runs attention for the block, and
immediately runs the output projection + DMA for the block's 4 row-tiles, so
PE-heavy projection work overlaps ACT-heavy softmax work of neighboring blocks.
"""

import sys

if "/opt/trn_rl_repo" not in sys.path:
    sys.path.insert(0, "/opt/trn_rl_repo")

from contextlib import ExitStack

import ml_dtypes
import numpy as np

import concourse.bacc as bacc
import concourse.tile as tile
from concourse import mybir
from concourse.bass_utils import run_bass_kernel_spmd

BF = ml_dtypes.bfloat16

B, T, D = 2, 2048, 2048
H, HK, HD = 16, 4, 128
REP = H // HK  # q heads per kv head (= heads per core)
P = 128
KC = D // P    # contraction chunks for the projections
NT = T // P    # 128-row tiles of T
NQB = T // 512 # 512-wide q blocks

_CACHE = {}


def _build(causal: bool):
    bf = mybir.dt.bfloat16
    f32 = mybir.dt.float32
    nc = bacc.Bacc("TRN2", target_bir_lowering=False, debug=False,
                   enable_asserts=False)

    xT = nc.dram_tensor("xT", [D, T], bf, kind="ExternalInput").ap()
    wq = nc.dram_tensor("wq", [D, REP * HD], bf, kind="ExternalInput").ap()
    wk = nc.dram_tensor("wk", [D, HD], bf, kind="ExternalInput").ap()
    wv = nc.dram_tensor("wv", [D, HD], bf, kind="ExternalInput").ap()
    wo = nc.dram_tensor("wo", [REP * HD, D], bf, kind="ExternalInput").ap()
    cos = nc.dram_tensor("cose", [P, T], bf, kind="ExternalInput").ap()
    sin = nc.dram_tensor("sine", [P, T], bf, kind="ExternalInput").ap()
    mt = nc.dram_tensor("mt", [P, P], bf, kind="ExternalInput").ap()
    idn = nc.dram_tensor("idn", [P, P], bf, kind="ExternalInput").ap()
    if causal:
        masks = nc.dram_tensor("masks", [P, 4 * 512], bf,
                               kind="ExternalInput").ap()
    else:
        maskT = nc.dram_tensor("maskT", [T, T], bf, kind="ExternalInput").ap()
    out = nc.dram_tensor("out", [T, D], bf, kind="ExternalOutput").ap()

    EXP = mybir.ActivationFunctionType.Exp

    with tile.TileContext(nc) as tc, ExitStack() as ctx:
        singles = ctx.enter_context(tc.tile_pool(name="singles", bufs=1))
        ps = ctx.enter_context(tc.tile_pool(name="ps", bufs=8, space="PSUM"))
        sb_raw = ctx.enter_context(tc.tile_pool(name="raw", bufs=3))
        sb_tmp = ctx.enter_context(tc.tile_pool(name="tmp", bufs=4))
        sb_probs = ctx.enter_context(tc.tile_pool(name="probs", bufs=8))
        sb_small = ctx.enter_context(tc.tile_pool(name="small", bufs=4))
        sb_out = ctx.enter_context(tc.tile_pool(name="outst", bufs=3))
        if not causal:
            sb_mask = ctx.enter_context(tc.tile_pool(name="mask", bufs=18))

        # ---- resident inputs ----
        # weights first (small, needed by the first matmuls), xT chunks
        # alternating between the two HWDGE queues (SP / Activation).
        wk_sb = singles.tile([P, KC, HD], bf, tag="wk")
        nc.sync.dma_start(out=wk_sb, in_=wk.rearrange("(c p) n -> p c n", p=P))
        wq_sb = singles.tile([P, KC, REP * HD], bf, tag="wq")
        nc.scalar.dma_start(out=wq_sb,
                            in_=wq.rearrange("(c p) n -> p c n", p=P))
        wv_sb = singles.tile([P, KC, HD], bf, tag="wv")
        nc.sync.dma_start(out=wv_sb, in_=wv.rearrange("(c p) n -> p c n", p=P))
        cos_sb = singles.tile([P, T], bf, tag="cos")
        nc.scalar.dma_start(out=cos_sb, in_=cos)
        sin_sb = singles.tile([P, T], bf, tag="sin")
        nc.scalar.dma_start(out=sin_sb, in_=sin)
        mt_sb = singles.tile([P, P], bf, tag="mt")
        nc.sync.dma_start(out=mt_sb, in_=mt)
        xT_t = [[None, None] for _ in range(KC)]
        for cb in range(2):
            for c in range(KC):
                t_ = singles.tile([P, 1024], bf, tag=f"xT{c}_{cb}",
                                  name=f"xT{c}_{cb}")
                eng = nc.sync if c % 2 == 0 else nc.scalar
                eng.dma_start(
                    out=t_, in_=xT[c * P:(c + 1) * P,
                                   cb * 1024:(cb + 1) * 1024])
                xT_t[c][cb] = t_

        def xsl(c, col0, width):
            cb = col0 // 1024
            off = col0 - cb * 1024
            return xT_t[c][cb][:, off:off + width]

        wo_sb = singles.tile([P, REP, D], bf, tag="wo")
        nc.sync.dma_start(out=wo_sb,
                          in_=wo.rearrange("(h p) d -> p h d", p=P))

        id_sb = singles.tile([P, P], bf, tag="idn")
        nc.scalar.dma_start(out=id_sb, in_=idn)
        if causal:
            # masks_sb[s, r, q] = 1.0 if r*128 + s <= q else 0.0
            masks_sb = singles.tile([P, 4, 512], bf, tag="masks")
            nc.scalar.dma_start(out=masks_sb, in_=masks.rearrange(
                "p (r n) -> p r n", r=4))

        qT = singles.tile([P, REP, T], bf, tag="qT")
        kT = singles.tile([P, T], bf, tag="kT")
        vax = singles.tile([P, NT, HD + 1], bf, tag="vax")
        oT = singles.tile([P, REP, T], bf, tag="oT")
        nc.vector.memset(vax[:, :, HD], 1.0)

        def proj_rope(dst_slice, lhsT_of, nb, tag):
            # dst_slice: bf16 [P, 512] target; lhsT_of(c) -> [P(Dchunk), 128]
            sl = slice(nb * 512, (nb + 1) * 512)
            pt = ps.tile([P, 512], f32, tag="ps", name=f"pjps{tag}{nb}")
            for c in range(KC):
                nc.tensor.matmul(pt, lhsT=lhsT_of(c),
                                 rhs=xsl(c, nb * 512, 512),
                                 start=(c == 0), stop=(c == KC - 1))
            raw = sb_raw.tile([P, 512], bf, tag="raw", name=f"raw{tag}{nb}")
            # psum->sbuf staging split between ACT and DVE
            if tag in ("k", "q0", "q2"):
                nc.scalar.copy(raw, pt)
            else:
                nc.vector.tensor_copy(raw, pt)
            sh = ps.tile([P, 512], f32, tag="ps", name=f"shps{tag}{nb}")
            nc.tensor.matmul(sh, lhsT=mt_sb, rhs=raw, start=True, stop=True)
            ta = sb_tmp.tile([P, 512], bf, tag="tmp", name=f"ta{tag}{nb}")
            nc.vector.tensor_mul(ta, raw, cos_sb[:, sl])
            tb = sb_tmp.tile([P, 512], bf, tag="tmp", name=f"tb{tag}{nb}")
            nc.vector.tensor_mul(tb, sh, sin_sb[:, sl])
            nc.vector.tensor_add(dst_slice, ta, tb)

        # ---- fused pipeline over 512-wide q blocks ----
        for qb in range(NQB):
            qsl = slice(qb * 512, (qb + 1) * 512)
            # -- projections for this block: k, v (packed), q (4 heads) --
            proj_rope(kT[:, qsl], lambda c: wk_sb[:, c], qb, "k")
            for mi in range(4):
                m = qb * 4 + mi
                pv = ps.tile([P, P], f32, tag="ps", name=f"vps{qb}_{mi}")
                for c in range(KC):
                    nc.tensor.matmul(pv, lhsT=xsl(c, m * P, P),
                                     rhs=wv_sb[:, c],
                                     start=(c == 0), stop=(c == KC - 1))
                nc.vector.tensor_copy(vax[:, m, :HD], pv)
            for h in range(REP):
                proj_rope(qT[:, h, qsl],
                          lambda c, h=h: wq_sb[:, c, h * HD:(h + 1) * HD],
                          qb, f"q{h}")

            # -- attention for this block --
            nj = 4 * qb + 4 if causal else NT
            if not causal:
                mts = []
                for j in range(nj):
                    t_ = sb_mask.tile([P, 512], bf, tag="maskt",
                                      name=f"mk{qb}_{j}")
                    nc.sync.dma_start(
                        out=t_, in_=maskT[j * P:(j + 1) * P, qsl])
                    mts.append(t_)
            for h in range(REP):
                # out_aug accumulators packed 2 per PSUM bank
                oaug = [ps.tile([P, HD + 1], f32, tag="ps",
                                name=f"oa{qb}_{h}_{k}") for k in range(4)]
                for j in range(nj):
                    r = j - 4 * qb if causal else -1
                    q0 = max(r, 0) * P  # first valid q column in this block
                    sc = ps.tile([P, 512], f32, tag="ps",
                                 name=f"sc{qb}_{h}_{j}")
                    nc.tensor.matmul(sc[:, q0:], lhsT=kT[:, j * P:(j + 1) * P],
                                     rhs=qT[:, h, qb * 512 + q0:(qb + 1) * 512],
                                     start=True, stop=True)
                    if not causal:
                        nc.vector.tensor_add(sc, sc, mts[j])
                    pr = sb_probs.tile([P, 512], bf, tag="probs",
                                       name=f"pr{qb}_{h}_{j}")
                    nc.scalar.activation(pr[:, q0:], sc[:, q0:], EXP)
                    if causal and r >= 0:
                        nc.vector.tensor_mul(pr[:, q0:], pr[:, q0:],
                                             masks_sb[:, r, q0:])
                    for mi in range(4):
                        m = qb * 4 + mi
                        if causal and j > m:
                            continue
                        last = (j == m) if causal else (j == nj - 1)
                        nc.tensor.matmul(oaug[mi],
                                         lhsT=pr[:, mi * P:(mi + 1) * P],
                                         rhs=vax[:, j, :],
                                         start=(j == 0), stop=last)
                for mi in range(4):
                    m = qb * 4 + mi
                    rec = sb_small.tile([P, 1], f32, tag="rec",
                                        name=f"rc{qb}_{h}_{mi}")
                    nc.vector.reciprocal(rec, oaug[mi][:, HD:HD + 1])
                    on = sb_small.tile([P, HD], bf, tag="onrm",
                                       name=f"on{qb}_{h}_{mi}")
                    nc.vector.tensor_scalar_mul(on, oaug[mi][:, :HD], rec)
                    tp = ps.tile([P, P], bf, tag="ps",
                                 name=f"tp{qb}_{h}_{mi}")
                    nc.tensor.transpose(tp, on, id_sb)
                    nc.vector.tensor_copy(oT[:, h, m * P:(m + 1) * P], tp)

            # -- output projection for this block's 4 row-tiles --
            for mi in range(4):
                m = qb * 4 + mi
                ost = sb_out.tile([P, D], bf, tag="outst", name=f"ost{m}")
                for n in range(D // 512):
                    wops = ps.tile([P, 512], f32, tag="ps",
                                   name=f"wops{m}_{n}")
                    for h in range(REP):
                        nc.tensor.matmul(
                            wops, lhsT=oT[:, h, m * P:(m + 1) * P],
                            rhs=wo_sb[:, h, n * 512:(n + 1) * 512],
                            start=(h == 0), stop=(h == REP - 1))
                    if n == 3:
                        nc.scalar.copy(ost[:, n * 512:(n + 1) * 512], wops)
                    else:
                        nc.vector.tensor_copy(
                            ost[:, n * 512:(n + 1) * 512], wops)
                eng = nc.sync if m % 2 == 0 else nc.scalar
                eng.dma_start(out=out[m * P:(m + 1) * P, :], in_=ost)

    nc.compile()
    return nc


def _get(causal: bool):
    if causal not in _CACHE:
        _CACHE[causal] = _build(causal)
    return _CACHE[causal]


def _is_causal(mask: np.ndarray) -> bool:
    if mask.shape != (T, T):
        return False
    tril = np.tril(np.ones((T, T), dtype=bool))
    if not np.all(mask[tril] == 0.0):
        return False
    return bool(np.all(np.isneginf(mask[~tril])))


def kernel(x, freqs_cos, freqs_sin, mask, wq, wk, wv, wo):
    causal = _is_causal(np.asarray(mask))
    nc = _get(causal)

    scale = np.float32(1.0 / np.sqrt(HD))
    cos_e = np.repeat(np.ascontiguousarray(freqs_cos.T), 2, axis=0).astype(BF)
    sin_e = np.repeat(np.ascontiguousarray(freqs_sin.T), 2, axis=0).astype(BF)
    mt = np.zeros((P, P), BF)
    for i in range(P // 2):
        mt[2 * i + 1, 2 * i] = -1.0  # shuf[2i]   = -q[2i+1]
        mt[2 * i, 2 * i + 1] = 1.0   # shuf[2i+1] = +q[2i]

    idn = np.eye(P, dtype=BF)
    if causal:
        s_i = np.arange(P)[:, None]
        q_i = np.arange(512)[None, :]
        m_r = np.stack(
            [(r * P + s_i <= q_i) for r in range(4)], axis=1).astype(BF)
        masks_h = np.ascontiguousarray(m_r.reshape(P, 4 * 512))
    xT_b = [np.ascontiguousarray(x[b].T).astype(BF) for b in range(B)]
    in_maps = []
    for b in range(B):
        for g in range(HK):
            m = {
                "xT": xT_b[b],
                "wq": (wq[:, g * REP * HD:(g + 1) * REP * HD]
                       * scale).astype(BF),
                "wk": wk[:, g * HD:(g + 1) * HD].astype(BF),
                "wv": wv[:, g * HD:(g + 1) * HD].astype(BF),
                "wo": wo[g * REP * HD:(g + 1) * REP * HD, :].astype(BF),
                "cose": cos_e, "sine": sin_e, "mt": mt, "idn": idn,
            }
            if causal:
                m["masks"] = masks_h
            else:
                m["maskT"] = np.ascontiguousarray(mask.T).astype(BF)
            in_maps.append(m)

    res = run_bass_kernel_spmd(nc, in_maps, core_ids=list(range(B * HK)))
    full = np.zeros((B, T, D), np.float32)
    for b in range(B):
        for g in range(HK):
            full[b] += res.results[b * HK + g]["out"].astype(np.float32)
    return full
